# revision 20
# baseline (speedup 1.0000x reference)
"""Trainium2 Bass kernel for nn_AttnBlockpp3d_old (GroupNorm + 4-head spatial
self-attention + residual), data-parallel over batch across 8 NeuronCores.

Shapes (hardcoded): x [16, 256, 32, 32] f32, 4 nin weights [256, 256] + biases,
gn scale/bias [256]. Each core processes 2 batches of [256, 1024].

v2 structure (per core):
- phase 1 (both batches up front): bn_stats directly on the x tiles (x loaded
  once), group-combine + channel-broadcast via indicator matmuls, rsqrt via
  DVE bit-hack + Newton (no ScalarE table switching; the exp set stays
  resident the whole kernel and is preloaded during the DMA phase).
- q/k projections accumulate into [128, 1024] 2-bank PSUM tiles (k stationary
  reused across the two 512-wide halves); v produced transposed with b2
  folded in as a K=1 ones matmul.
- attention j-outer: scores for both heads row-tiled concurrent into
  [128, 1024] PSUM tiles; softmax exp as ONE [128, 1024] call per (j, head).
  Exp runs on ScalarE (table exp) for most tiles and on VectorE for a tunable
  subset via the Schraudolph bit-hack: int16(x*C1+C2) bit-viewed as bf16
  (max ~3% per-element, ~7e-4 end-to-end).
- softmax denominator rides A@V as a ones-column; normalization: reciprocal
  of the denominator row, partition-broadcast via a tiny fp32 ones matmul on
  the PE (no DRAM bounce), one [128, 1024] multiply.
- final nin adds b3 via a K=1 matmul; residual rides the PSUM->SBUF move.
"""

import numpy as np

N_CORES = 8
B_TOTAL = 16
B_PER_CORE = B_TOTAL // N_CORES
C = 256
H = 32
S = H * H          # 1024 spatial positions (N_FRAMES=1)
NG = 32            # groupnorm groups -> 8 channels/group
NH = 4             # heads
CH = C // NH       # 64 channels/head
EPS = 1e-6
SCALE = CH ** -0.5  # 0.125

# Schraudolph exp for DVE-offloaded tiles: bf16 bits = x*EC1 + EC2 (round),
# with the 1/sqrt(64) score scale folded into EC1.
EC1 = SCALE * 128.0 / float(np.log(2.0))
EC2 = 16250.25

# (hp, j) exp tiles computed on VectorE instead of ScalarE (load balance).
DVE_EXP = {(0, 1), (0, 4), (0, 7), (1, 0), (1, 2), (1, 5)}

_CACHE: dict = {}


def _build_nc(debug_taps=False):
    from contextlib import ExitStack

    import concourse.bacc as bacc
    import concourse.bass as bass
    import concourse.mybir as mybir
    import concourse.tile as tile

    fp32 = mybir.dt.float32
    bf16 = mybir.dt.bfloat16
    i16 = mybir.dt.int16
    i32 = mybir.dt.int32
    AF = mybir.ActivationFunctionType
    OP = mybir.AluOpType
    ts = bass.ts

    nc = bacc.Bacc("TRN2")

    x_d = nc.dram_tensor("x", [B_PER_CORE, C, S], fp32, kind="ExternalInput")
    gns_d = nc.dram_tensor("gn_scale", [C], fp32, kind="ExternalInput")
    gnb_d = nc.dram_tensor("gn_bias", [C], fp32, kind="ExternalInput")
    W_d = [nc.dram_tensor(f"W{i}", [C, C], fp32, kind="ExternalInput") for i in range(4)]
    b_d = [nc.dram_tensor(f"b{i}", [C], fp32, kind="ExternalInput") for i in range(4)]
    y_d = nc.dram_tensor("y", [B_PER_CORE, C, S], fp32, kind="ExternalOutput")
    dbg = {}
    if debug_taps:
        for nm, shp, dt_ in (("h", [2, 128, S], bf16), ("q", [2, 128, S], bf16),
                             ("k", [2, 128, S], bf16), ("vt0", [128, NH, CH + 1], bf16),
                             ("e00", [128, S], bf16), ("den0", [2, S], fp32),
                             ("hh0", [128, S], bf16), ("ab", [2, 128, 2], fp32)):
            dbg[nm] = nc.dram_tensor(f"dbg_{nm}", shp, dt_, kind="ExternalOutput")

    with tile.TileContext(nc) as tc, ExitStack() as ctx:
        const = ctx.enter_context(tc.tile_pool(name="const", bufs=1))
        stage = ctx.enter_context(tc.tile_pool(name="stage", bufs=2))
        xpool = ctx.enter_context(tc.tile_pool(name="xpool", bufs=2))
        hpool = ctx.enter_context(tc.tile_pool(name="hpool", bufs=2))
        vpool = ctx.enter_context(tc.tile_pool(name="vpool", bufs=18))
        epool = ctx.enter_context(tc.tile_pool(name="epool", bufs=8))
        rpool = ctx.enter_context(tc.tile_pool(name="rpool", bufs=2))
        spool = ctx.enter_context(tc.tile_pool(name="spool", bufs=3))

        # PSUM (8 banks): s00/s01/s10/s11 = [128,512] score tiles
        # (head x double-buffer), h0/h1 = [65,512] hh accumulators (also
        # vt/stats scratch), aux = [128,1024] 2-bank slot (qk projections /
        # rdb broadcast / fin / warm-up).
        ps = ctx.enter_context(tc.tile_pool(name="ps", bufs=1, space="PSUM"))

        # ---- phase 0: loads + constants ----
        # x loads first (stats are on the critical path)
        xs = []
        for b in range(B_PER_CORE):
            x_sb = []
            for ct in range(2):
                t = xpool.tile([128, S], fp32, tag=f"x{b}{ct}", name=f"x_sb{b}{ct}")
                nc.sync.dma_start(out=t, in_=x_d[b, ts(ct, 128), :])
                x_sb.append(t)
            xs.append(x_sb)

        # W0..W3 as bf16 [128, c_tile 2, d 256] (partition p = channel p + 128*ct)
        Wstage, Wsb_t = [], []
        for i in range(4):
            st = stage.tile([128, 2, C], fp32, tag=f"wstage{i}")
            nc.sync.dma_start(out=st, in_=W_d[i].rearrange("(a p) d -> p a d", p=128))
            Wstage.append(st)
            wt = const.tile([128, 2, C], bf16, tag=f"w{i}")
            Wsb_t.append(wt)
        # W casts emitted after the stats chains (keeps both the DVE queue
        # and the GpSimd SWDGE clear early for stats and DMA descriptors).
        Wsb = [[Wsb_t[i][:, ct, :] for ct in range(2)] for i in range(4)]

        def col_tiles(dram, name):
            out = []
            for ct in range(2):
                t = const.tile([128, 1], fp32, tag=f"{name}{ct}")
                nc.sync.dma_start(out=t, in_=dram[ts(ct, 128)][:, None])
                out.append(t)
            return out

        gns_sb = col_tiles(gns_d, "gns")
        gnb_sb = col_tiles(gnb_d, "gnb")
        b0_sb = col_tiles(b_d[0], "b0")
        b1_sb = col_tiles(b_d[1], "b1")

        # b2 / b3 as single-partition rows in bf16 (K=1 matmul operands)
        b2st = stage.tile([1, C], fp32, tag="b2st")
        nc.sync.dma_start(out=b2st, in_=b_d[2][None, :])
        b2row = const.tile([1, C], bf16, tag="b2row")
        nc.vector.tensor_copy(out=b2row, in_=b2st)
        b3st = stage.tile([1, C], fp32, tag="b3st")
        nc.sync.dma_start(out=b3st, in_=b_d[3][None, :])
        b3row = const.tile([1, C], bf16, tag="b3row")
        nc.vector.tensor_copy(out=b3row, in_=b3st)

        ones1 = const.tile([1, 128], bf16, tag="ones1")
        nc.vector.memset(ones1, 1.0)
        ones512 = const.tile([1, 512], bf16, tag="ones512")
        nc.vector.memset(ones512, 1.0)

        # HAM warm-up + exp-table preload during the load phase.
        warm = const.tile([128, 256], bf16, tag="warm")
        nc.vector.memset(warm, 1.0)
        warm_ps = ps.tile([128, 256], fp32, tag="aux", bufs=2, name="warm_ps")
        for i in range(24):
            nc.tensor.matmul(warm_ps, lhsT=warm[:, 0:128], rhs=warm,
                             start=True, stop=True)
        expwarm = const.tile([1, 8], fp32, tag="expwarm")
        nc.gpsimd.memset(expwarm, 0.0)
        expwarm2 = const.tile([1, 8], bf16, tag="expwarm2")
        nc.scalar.activation(out=expwarm2, in_=expwarm, func=AF.Exp, scale=1.0)

        # Q8a/Q8b [128, 32]: Q8a[p,g]=1 iff p//8==g (g<16); Q8b: g==p//8+16
        q8 = []
        for ct in range(2):
            t = const.tile([128, NG], fp32, tag=f"q8{ct}")
            nc.gpsimd.memset(t, 1.0)
            base = 128 * ct
            nc.gpsimd.affine_select(out=t, in_=t, compare_op=OP.is_ge, fill=0.0,
                                    pattern=[[-8, NG]], base=base,
                                    channel_multiplier=1)
            nc.gpsimd.affine_select(out=t, in_=t, compare_op=OP.is_ge, fill=0.0,
                                    pattern=[[8, NG]], base=7 - base,
                                    channel_multiplier=-1)
            q8.append(t)

        # Q2[ct] [32, 128]: Q2[g, c] = 1 iff group(ct*128 + c) == g
        q2 = []
        for ct in range(2):
            t = const.tile([NG, 128], fp32, tag=f"q2{ct}")
            nc.gpsimd.memset(t, 1.0)
            base = ct * 128
            nc.gpsimd.affine_select(out=t, in_=t, compare_op=OP.is_ge, fill=0.0,
                                    pattern=[[1, 128]], base=base, channel_multiplier=-8)
            nc.gpsimd.affine_select(out=t, in_=t, compare_op=OP.is_ge, fill=0.0,
                                    pattern=[[-1, 128]], base=7 - base, channel_multiplier=8)
            q2.append(t)

        # sel[h] [1, 128]: 1 iff p//64 == h  (denominator partition-broadcast)
        sel = []
        for hp in range(2):
            t = const.tile([1, 128], fp32, tag=f"sel{hp}")
            nc.gpsimd.memset(t, 1.0)
            if hp == 0:
                nc.gpsimd.affine_select(out=t, in_=t, compare_op=OP.is_ge,
                                        fill=0.0, pattern=[[-1, 128]], base=63,
                                        channel_multiplier=0)
            else:
                nc.gpsimd.affine_select(out=t, in_=t, compare_op=OP.is_ge,
                                        fill=0.0, pattern=[[1, 128]], base=-64,
                                        channel_multiplier=0)
            sel.append(t)

        # ---- phase 1: stats for both batches, then projections ----
        h_all = [None, None]
        qk_all, vt_all = [None, None], [None, None]

        def stats(b):
            x_sb = xs[b]
            rhs2 = []
            for ct in range(2):
                st6 = spool.tile([128, 2, 6], fp32, tag=f"st6{ct}", bufs=2)
                for i in range(2):
                    nc.vector.bn_stats(out=st6[:, i, :], in_=x_sb[ct][:, ts(i, 512)])
                mv = spool.tile([128, 2], fp32, tag=f"mv{ct}", bufs=2)
                nc.vector.bn_aggr(out=mv, in_=st6)
                r2 = spool.tile([128, 2], fp32, tag=f"rhs2{ct}", bufs=2)
                nc.vector.tensor_copy(out=r2[:, 0:1], in_=mv[:, 0:1])
                nc.vector.tensor_mul(out=r2[:, 1:2], in0=mv[:, 0:1], in1=mv[:, 0:1])
                nc.vector.tensor_add(out=r2[:, 1:2], in0=r2[:, 1:2], in1=mv[:, 1:2])
                rhs2.append(r2)
            gs_ps = ps.tile([NG, 2], fp32, tag="h1", name="gs_ps")
            nc.tensor.matmul(gs_ps, lhsT=q8[0], rhs=rhs2[0], start=True, stop=False)
            nc.tensor.matmul(gs_ps, lhsT=q8[1], rhs=rhs2[1], start=False, stop=True)
            gmv = spool.tile([NG, 2], fp32, tag="gmv", bufs=2)
            nc.vector.tensor_scalar_mul(out=gmv, in0=gs_ps, scalar1=0.125)
            veps = spool.tile([NG, 1], fp32, tag="veps", bufs=2)
            nc.vector.tensor_mul(out=veps, in0=gmv[:, 0:1], in1=gmv[:, 0:1])
            nc.vector.tensor_tensor(out=veps, in0=gmv[:, 1:2], in1=veps,
                                    op=OP.subtract)
            nc.vector.tensor_scalar_add(out=veps, in0=veps, scalar1=EPS)
            # rsqrt bit-hack + 3 Newton iterations (all on DVE)
            ri = spool.tile([NG, 1], i32, tag="ri", bufs=2)
            nc.vector.tensor_scalar(out=ri, in0=veps.bitcast(i32), scalar1=1,
                                    scalar2=None, op0=OP.logical_shift_right)
            ri2 = spool.tile([NG, 1], i32, tag="ri2", bufs=2)
            nc.vector.tensor_scalar(out=ri2, in0=ri, scalar1=-1,
                                    scalar2=0x5F3759DF, op0=OP.mult, op1=OP.add)
            cur = ri2.bitcast(fp32)
            nt = spool.tile([NG, 1], fp32, tag="nt", bufs=2)
            for it in range(3):
                nc.vector.tensor_tensor(out=nt, in0=cur, in1=cur, op=OP.mult)
                nc.vector.tensor_tensor(out=nt, in0=nt, in1=veps, op=OP.mult)
                nc.vector.tensor_scalar(out=nt, in0=nt, scalar1=-0.5, scalar2=1.5,
                                        op0=OP.mult, op1=OP.add)
                dst = spool.tile([NG, 1], fp32, tag=f"ny{it}", bufs=2)
                nc.vector.tensor_tensor(out=dst, in0=cur, in1=nt, op=OP.mult)
                cur = dst
            ab_g = spool.tile([NG, 2], fp32, tag="abg", bufs=2)
            nc.vector.tensor_copy(out=ab_g[:, 0:1], in_=cur)
            nc.vector.tensor_mul(out=ab_g[:, 1:2], in0=gmv[:, 0:1], in1=cur)
            nc.vector.tensor_scalar_mul(out=ab_g[:, 1:2], in0=ab_g[:, 1:2],
                                        scalar1=-1.0)
            h_bf = []
            for ct in range(2):
                ab_ps = ps.tile([128, 2], fp32, tag="h1", name="ab_ps")
                nc.tensor.matmul(ab_ps, lhsT=q2[ct], rhs=ab_g, start=True, stop=True)
                AB = spool.tile([128, 2], fp32, tag=f"AB{ct}", bufs=2)
                nc.vector.tensor_mul(out=AB[:, 0:1], in0=ab_ps[:, 0:1], in1=gns_sb[ct])
                nc.vector.tensor_mul(out=AB[:, 1:2], in0=ab_ps[:, 1:2], in1=gns_sb[ct])
                nc.vector.tensor_add(out=AB[:, 1:2], in0=AB[:, 1:2], in1=gnb_sb[ct])
                ht = hpool.tile([128, S], bf16, tag=f"h{ct}", bufs=2)
                nc.vector.tensor_scalar(out=ht, in0=x_sb[ct],
                                        scalar1=AB[:, 0:1], scalar2=AB[:, 1:2],
                                        op0=OP.mult, op1=OP.add)
                if debug_taps and b == 0:
                    nc.sync.dma_start(out=dbg["h"][ct], in_=ht)
                    nc.sync.dma_start(out=dbg["ab"][ct], in_=AB)
                h_bf.append(ht)
            h_all[b] = h_bf

        def proj(b):
            h_bf = h_all[b]
            qk_sb = [[None, None], [None, None]]
            for p, bias in ((0, b0_sb), (1, b1_sb)):
                for dt in range(2):
                    t = hpool.tile([128, S], bf16, tag=f"qk{p}{dt}", bufs=2)
                    for sc in range(2):
                        qk_ps = ps.tile([128, 512], fp32, tag="aux", bufs=2,
                                        name="qk_ps")
                        for ct in range(2):
                            nc.tensor.matmul(
                                qk_ps,
                                lhsT=Wsb[p][ct][:, ts(dt, 128)],
                                rhs=h_bf[ct][:, ts(sc, 512)],
                                start=(ct == 0), stop=(ct == 1))
                        if b == 0:
                            nc.scalar.activation(out=t[:, ts(sc, 512)],
                                                 in_=qk_ps, func=AF.Identity,
                                                 bias=bias[dt], scale=1.0)
                        else:
                            nc.vector.tensor_scalar_add(out=t[:, ts(sc, 512)],
                                                        in0=qk_ps,
                                                        scalar1=bias[dt])
                    if debug_taps and b == 0:
                        nc.sync.dma_start(out=dbg["q" if p == 0 else "k"][dt], in_=t)
                    qk_sb[p][dt] = t

            vt_tiles = []
            for j in range(8):
                vt_ps = ps.tile([128, C], fp32, tag=f"h{j % 2}", name="vt_ps")
                nc.tensor.matmul(vt_ps, lhsT=h_bf[0][:, ts(j, 128)],
                                 rhs=Wsb[2][0], start=True, stop=False)
                nc.tensor.matmul(vt_ps, lhsT=h_bf[1][:, ts(j, 128)],
                                 rhs=Wsb[2][1], start=False, stop=False)
                nc.tensor.matmul(vt_ps, lhsT=ones1, rhs=b2row,
                                 start=False, stop=True)
                vt = vpool.tile([128, NH, CH + 1], bf16, tag="vt")
                nc.gpsimd.memset(vt[:, :, CH:CH + 1], 1.0)
                if b == 0:
                    nc.scalar.activation(
                        out=vt[:, :, 0:CH],
                        in_=vt_ps.rearrange("p (h c) -> p h c", h=NH),
                        func=AF.Identity, scale=1.0)
                else:
                    nc.vector.tensor_copy(
                        out=vt[:, :, 0:CH],
                        in_=vt_ps.rearrange("p (h c) -> p h c", h=NH))
                if debug_taps and b == 0 and j == 0:
                    nc.sync.dma_start(out=dbg["vt0"][:, :, :], in_=vt)
                vt_tiles.append(vt)
            qk_all[b] = qk_sb
            vt_all[b] = vt_tiles

        stats(0)
        nc.vector.tensor_copy(out=Wsb_t[0], in_=Wstage[0])
        nc.vector.tensor_copy(out=Wsb_t[1], in_=Wstage[1])
        stats(1)
        nc.vector.tensor_copy(out=Wsb_t[2], in_=Wstage[2])
        proj(0)
        nc.vector.tensor_copy(out=Wsb_t[3], in_=Wstage[3])

        # ---- phase 2: attention as an 8-unit pipeline ----
        # One unit = (batch, head-pair, t-block): its 8-j score/exp/A@V loop,
        # then (per unit) denominator reciprocal + broadcast + normalization.
        # The denominator only involves this unit's t-block, so norm and fin
        # pipeline at unit granularity -- the tail is one unit's chain, and
        # the PE queue always holds the next unit's matmuls (keeps HAM warm).
        hh_t_all = {}
        hh_u65_all = {}
        norm_rdb = {}
        dpool = ctx.enter_context(tc.tile_pool(name="dpool", bufs=4, space="DRAM"))

        def jloop(b, pr, sc):
            qk_sb = qk_all[b]
            vt_tiles = vt_all[b]
            hh_ps = [ps.tile([CH + 1, 512], fp32, tag=f"h{hp}",
                             name=f"hh_ps{hp}") for hp in range(2)]
            hh_u65 = [rpool.tile([CH + 1, 512], fp32, tag=f"hhu{hp}", bufs=3,
                                 name=f"hh_u65{hp}") for hp in range(2)]
            hh_u65_all[(b, pr, sc)] = hh_u65
            # software-pipelined emission: A@V for j-1 is emitted after the
            # scores for j, so the PE queue always holds ready matmuls
            # (back-to-back rate instead of isolated-matmul rate).
            pend = None
            for j in range(8):
                ets = [None, None]
                for hp in range(2):
                    s_ps = ps.tile([128, 512], fp32,
                                   tag=f"s{hp}{j % 2}", name="s_ps")
                    nc.tensor.matmul(
                        s_ps,
                        lhsT=qk_sb[1][pr][ts(hp, CH), ts(j, 128)],
                        rhs=qk_sb[0][pr][ts(hp, CH), ts(sc, 512)],
                        start=True, stop=True)
                    if (hp, j) in DVE_EXP:
                        ei = epool.tile([128, 512], i16, tag="ei")
                        nc.vector.tensor_scalar(out=ei, in0=s_ps,
                                                scalar1=EC1, scalar2=EC2,
                                                op0=OP.mult, op1=OP.add)
                        et = ei.bitcast(bf16)
                    else:
                        et = epool.tile([128, 512], bf16, tag="e")
                        nc.scalar.activation(out=et, in_=s_ps,
                                             func=AF.Exp, scale=SCALE)
                    if debug_taps and b == 0 and pr == 0 and j == 0 and hp == 0:
                        nc.sync.dma_start(out=dbg["e00"][:, ts(sc, 512)], in_=et)
                    ets[hp] = et
                if pend is not None:
                    pj, pets = pend
                    for hp in range(2):
                        nc.tensor.matmul(
                            hh_ps[hp],
                            lhsT=vt_tiles[pj][:, 2 * pr + hp, :],
                            rhs=pets[hp],
                            start=(pj == 0), stop=False)
                pend = (j, ets)
            pj, pets = pend
            for hp in range(2):
                nc.tensor.matmul(
                    hh_ps[hp],
                    lhsT=vt_tiles[pj][:, 2 * pr + hp, :],
                    rhs=pets[hp],
                    start=False, stop=True)
            # evict hh+den (ScalarE; frees the hh accumulator banks)
            for hp in range(2):
                nc.scalar.activation(out=hh_u65[hp], in_=hh_ps[hp],
                                     func=AF.Identity, scale=1.0)

        def normpre(b, pr, sc):
            # den row -> partition 0 (custom-DVE needs partition-0 input),
            # recip, DRAM-bounce partition-broadcast (all off the PE)
            hh_u65 = hh_u65_all[(b, pr, sc)]
            den0 = [rpool.tile([1, 512], fp32, tag=f"den{hp}", bufs=2,
                               name=f"den0{hp}") for hp in range(2)]
            rdp = [rpool.tile([1, 512], fp32, tag=f"rdp{hp}", bufs=2,
                              name=f"rdp{hp}") for hp in range(2)]
            rdb = [rpool.tile([CH, 512], fp32, tag=f"rdb{hp}", bufs=2,
                              name=f"rdb{hp}") for hp in range(2)]
            for hp in range(2):
                nc.sync.dma_start(out=den0[hp], in_=hh_u65[hp][CH:CH + 1, :])
                nc.vector.reciprocal_approx_fast(out=rdp[hp], in_=den0[hp])
                rdd = dpool.tile([1, 512], fp32, tag=f"rdd{hp}")
                nc.sync.dma_start(out=rdd, in_=rdp[hp])
                nc.sync.dma_start(out=rdb[hp], in_=rdd.to_broadcast([CH, 512]))
            if debug_taps and b == 0 and pr == 0 and sc == 0:
                nc.sync.dma_start(out=dbg["den0"][0:1, 0:512], in_=rdp[0])
                nc.sync.dma_start(out=dbg["den0"][1:2, 0:512], in_=rdp[1])
            norm_rdb[(b, pr, sc)] = rdb

        def normpost(b, pr, sc):
            hh_u65 = hh_u65_all[(b, pr, sc)]
            rdb = norm_rdb[(b, pr, sc)]
            hh_t = hpool.tile([128, 512], bf16, tag="hh", bufs=6)
            for hp in range(2):
                nc.vector.tensor_tensor(out=hh_t[ts(hp, CH), :],
                                        in0=hh_u65[hp][0:CH, :],
                                        in1=rdb[hp],
                                        op=OP.mult)
            if debug_taps and b == 0 and pr == 0:
                nc.sync.dma_start(out=dbg["hh0"][:, ts(sc, 512)], in_=hh_t)
            hh_t_all[(b, pr, sc)] = hh_t

        out_t_all = {}

        def fin(b, sc):
            x_sb = xs[b]
            for dt in range(2):
                if (b, dt) not in out_t_all:
                    out_t_all[(b, dt)] = xpool.tile([128, S], fp32,
                                                    tag=f"out{dt}", bufs=2,
                                                    name=f"out{dt}")
                out_t = out_t_all[(b, dt)]
                fin_ps = ps.tile([128, 512], fp32, tag="aux", bufs=2,
                                 name="fin_ps")
                for ct in range(2):
                    nc.tensor.matmul(
                        fin_ps,
                        lhsT=Wsb[3][ct][:, ts(dt, 128)],
                        rhs=hh_t_all[(b, ct, sc)],
                        start=(ct == 0), stop=False)
                nc.tensor.matmul(fin_ps,
                                 lhsT=b3row[:, ts(dt, 128)], rhs=ones512,
                                 start=False, stop=True)
                nc.vector.tensor_add(out=out_t[:, ts(sc, 512)],
                                     in0=fin_ps,
                                     in1=x_sb[dt][:, ts(sc, 512)])
                if sc == 1:
                    nc.sync.dma_start(out=y_d[b, ts(dt, 128), :], in_=out_t)

        U = [(0, 0, 0), (0, 1, 0), (0, 0, 1), (0, 1, 1),
             (1, 0, 0), (1, 1, 0), (1, 0, 1), (1, 1, 1)]
        FIN_AFTER = {2: (0, 0), 4: (0, 1), 6: (1, 0)}

        for u, unit in enumerate(U):
            jloop(*unit)
            normpre(*unit)
            if u == 0:
                proj(1)
            if u >= 1:
                normpost(*U[u - 1])
            if u in FIN_AFTER:
                fb, fsc = FIN_AFTER[u]
                fin(fb, fsc)
        normpost(*U[7])
        fin(1, 1)

    nc.finalize()
    return nc


def _in_maps(inputs):
    x = np.ascontiguousarray(np.asarray(inputs["x"], dtype=np.float32))
    B = x.shape[0]
    xr = x.reshape(B, C, S)
    shared = {k: np.ascontiguousarray(np.asarray(inputs[k], dtype=np.float32))
              for k in ("gn_scale", "gn_bias", "W0", "b0", "W1", "b1", "W2", "b2",
                        "W3", "b3")}
    maps = []
    for core in range(N_CORES):
        m = dict(shared)
        m["x"] = np.ascontiguousarray(xr[core * B_PER_CORE:(core + 1) * B_PER_CORE])
        maps.append(m)
    return maps


def kernel(**inputs: np.ndarray) -> np.ndarray:
    from concourse.bass_utils import run_bass_kernel_spmd

    if "nc" not in _CACHE:
        _CACHE["nc"] = _build_nc()
    res = run_bass_kernel_spmd(_CACHE["nc"], _in_maps(inputs),
                               core_ids=list(range(N_CORES)))
    out = np.concatenate([res.results[c]["y"] for c in range(N_CORES)], axis=0)
    B = np.asarray(inputs["x"]).shape[0]
    return out.reshape(B, C, H, H).astype(np.float32)


def run_profiled(inputs):
    """Like kernel() but with trace=True; returns (out, exec_time_ns)."""
    from concourse.bass_utils import run_bass_kernel_spmd

    if "nc" not in _CACHE:
        _CACHE["nc"] = _build_nc()
    res = run_bass_kernel_spmd(_CACHE["nc"], _in_maps(inputs),
                               core_ids=list(range(N_CORES)), trace=True)
    out = np.concatenate([res.results[c]["y"] for c in range(N_CORES)], axis=0)
    B = np.asarray(inputs["x"]).shape[0]
    return out.reshape(B, C, H, H).astype(np.float32), res.exec_time_ns
